# revision 26
# baseline (speedup 1.0000x reference)
"""Trainium2 Bass kernel for nn_MultiHeadCausalTensionLayer.

Reference computation (B=1, T=2048, D=1024, H=16, HD=64, WN=64):
  q,k,v = x@wq, x@wk, x@wv  (per-head RoPE on q,k)
  scores[t,h,w] = q[t,h]·k[t-64+w,h] / 8          (w in [0,64), causal window)
  tau = sigmoid(scores) * causal_mask
  msg = (tau @ window_v) / clip(sum_w tau, 1e-6)
  out = rms_norm(x + msg.flat @ wo) * norm_scale

Sharding: sequence-parallel over T across 8 cores (256 rows each) with a
64-row halo; the halo is materialized host-side (zero-padded for core 0),
so each core's program is identical, fully local, and needs no collectives.

All matmuls run in bf16 (fp32 PSUM accumulation); weights are converted
host-side, activations are cast on-chip in the PSUM->SBUF copies. Per-head
attention is processed as head PAIRS: the two heads of a pair occupy SBUF
partition halves [0:64) / [64:128) of one 128-row chunk, so score matmuls
row-pack (K=64 tiles at array rows 0/64) and msg matmuls column-pack
(tile_position=(0,64) for the odd head). The tau row-sum ("mass") is
accumulated for all 16 heads into one [16, 256] PSUM tile via constant
one-hot lhsT matmuls, so the reciprocal runs once on 16 partitions instead
of per-head on one.
"""

import numpy as np

import concourse.bass as bass
import concourse.mybir as mybir
import concourse.tile as tile
from concourse import bacc, bass_utils

# Problem constants (hardcoded per harness contract).
B, T, D = 1, 2048, 1024
H, HD, WN = 16, 64, 64
ROPE_BASE = 10000.0
EPS = 1e-6
NCORES = 8
TLOC = T // NCORES          # 256 rows per core
TEXT = TLOC + WN            # 320 rows incl. halo
P = 128
KCH = D // P                # 8 contraction chunks
MCH = D // P                # 8 output chunks
NKB = TEXT // P + (1 if TEXT % P else 0)  # 3 key blocks (128,128,64)
NPAIR = H // 2              # 8 head pairs == output chunks

f32 = mybir.dt.float32
bf16 = mybir.dt.bfloat16


def _build_program(loop_reps=None, stage=40):
    nc = bacc.Bacc("TRN2", target_bir_lowering=False, debug=False)

    def din(name, shape, dt):
        return nc.dram_tensor(name, list(shape), dt, kind="ExternalInput").ap()

    x_halo = din("x_halo", (TEXT, D), f32)
    wq_d = din("wq_b", (D, D), bf16)
    wk_d = din("wk_b", (D, D), bf16)
    wv_d = din("wv_b", (D, D), bf16)
    wo_d = din("wo_b", (D, D), bf16)
    ident_d = din("ident", (P, P), f32)
    rotT_d = din("rotT", (P, P), bf16)
    cosq_d = din("cosq", (P, TLOC), bf16)
    sinq_d = din("sinq", (P, TLOC), f32)
    cosk_d = din("cosk", (P, TEXT), bf16)
    sink_d = din("sink", (P, TEXT), f32)
    masks_d = din("masks2", (P, NKB, 2, TLOC), bf16)
    emask_d = din("emaskT", (P, H, H), bf16)
    esel_d = din("esel", (H, MCH, P), bf16)
    nsc_d = din("norm_scale", (P, D), f32)
    y_d = nc.dram_tensor("y", [TLOC, D], f32, kind="ExternalOutput").ap()

    with tile.TileContext(nc) as tc:
        from contextlib import ExitStack
        with ExitStack() as ctx:
            if loop_reps is not None:
                loop = ctx.enter_context(tc.For_i(0, loop_reps, 1))
            sb = ctx.enter_context(tc.tile_pool(name="sb", bufs=1))
            sbw = ctx.enter_context(tc.tile_pool(name="sbw", bufs=3))
            sba = ctx.enter_context(tc.tile_pool(name="sba", bufs=2))
            sbt = ctx.enter_context(tc.tile_pool(name="sbt", bufs=2))
            sbz = ctx.enter_context(tc.tile_pool(name="sbz", bufs=2))
            # PSUM pools (bank-granular): pp 2 + prot 1 + ps 3 + pm 1 +
            # pmass 1 = 8 banks.
            # HW rule learned the hard way: matmuls on disjoint row groups
            # execute concurrently and FAULT when they write the same PSUM
            # bank. Within the score tile [P, kb, hh, t], head 0 (array rows
            # 0:64) lands in bank lower halves and head 1 (rows 64:128) in
            # upper halves; scores are issued hh-outer so the only
            # cross-row-group adjacency is (hh0,kb2)->(hh1,kb0) = different
            # banks. All other cross-row-group neighbors are full-row
            # matmuls, which serialize.
            pp = ctx.enter_context(tc.tile_pool(name="pp", bufs=2, space="PSUM"))
            prot = ctx.enter_context(tc.tile_pool(name="prot", bufs=1,
                                                  space="PSUM"))
            ps = ctx.enter_context(tc.tile_pool(name="ps", bufs=1, space="PSUM"))
            pm = ctx.enter_context(tc.tile_pool(name="pm", bufs=1, space="PSUM"))
            pms = ctx.enter_context(tc.tile_pool(name="pms", bufs=1, space="PSUM"))

            # ---- input DMAs (issue in consumption order) ----
            xe_t = sb.tile([P, NKB, D], f32, tag="xe")
            nc.sync.dma_start(xe_t[:, 0, :], x_halo[0:P])
            rot_t = sb.tile([P, P], bf16, tag="rot")
            nc.sync.dma_start(rot_t[:], rotT_d)
            ident_t = sb.tile([P, P], f32, tag="ident")
            nc.sync.dma_start(ident_t[:], ident_d)
            nc.sync.dma_start(xe_t[:, 1, :], x_halo[P:2 * P])
            nc.sync.dma_start(xe_t[0:TEXT - 2 * P, 2, :], x_halo[2 * P:TEXT])

            # ---- PE warm-up: dummy matmuls on a memset tile (no DMA
            # dependency), so the HAM clock gate reaches 8/8 before the
            # real work and the PE spins while input DMAs land ----
            wuin = sb.tile([P, P], bf16, tag="wuin")
            nc.vector.memset(wuin[:], 0.5)
            wu_t = pp.tile([P, 512], f32, tag="pp", name="warmup")
            for _ in range(32):
                nc.tensor.matmul(wu_t[:, 0:P], wuin[:], wuin[:],
                                 start=True, stop=True)

            def load_w(wd):
                wt = sbw.tile([P, KCH, D], bf16, tag="w")
                wr = wd.rearrange("(k p) d -> p k d", p=P)
                for g in range(4):
                    nc.gpsimd.dma_start(wt[:, 2 * g:2 * g + 2, :],
                                        wr[:, 2 * g:2 * g + 2, :])
                return wt

            cq_t = sb.tile([P, TLOC], bf16, tag="cq")
            sq_t = sb.tile([P, TLOC], f32, tag="sq")
            ck_t = sb.tile([P, TEXT], bf16, tag="ck")
            sk_t = sb.tile([P, TEXT], f32, tag="sk")
            nc.gpsimd.dma_start(cq_t[:], cosq_d)
            nc.gpsimd.dma_start(sq_t[:], sinq_d)
            nc.gpsimd.dma_start(ck_t[:], cosk_d)
            nc.gpsimd.dma_start(sk_t[:], sink_d)
            wv_t = load_w(wv_d)
            wq_t = load_w(wq_d)
            mask_t = sb.tile([P, NKB, 2, TLOC], bf16, tag="mask")
            nc.gpsimd.dma_start(mask_t[:], masks_d)
            emask_t = sb.tile([P, H, H], bf16, tag="emask")
            nc.gpsimd.dma_start(emask_t[:], emask_d)
            esel_t = sb.tile([H, MCH, P], bf16, tag="esel")
            nc.gpsimd.dma_start(esel_t[:], esel_d)
            wk_t = load_w(wk_d)
            xo_t = sb.tile([P, 2, D], f32, tag="xo")
            nc.scalar.dma_start(xo_t[:], x_halo[WN:].rearrange(
                "(c p) d -> p c d", p=P))
            nsc_t = sb.tile([P, D], f32, tag="nsc")
            nc.gpsimd.dma_start(nsc_t[:], nsc_d)
            wo_t = load_w(wo_d)

            # ---- transpose x -> xT [dout, text] bf16 ----
            xT_t = sb.tile([P, KCH, TEXT], bf16, tag="xT")
            for tc3 in range(NKB):
                rows = P if tc3 < 2 else TEXT - 2 * P
                for g in range(2):
                    pt = pp.tile([P, 4, P], f32, tag="pp")
                    for j in range(4):
                        nc.tensor.transpose(
                            pt[:, j, 0:rows],
                            xe_t[0:rows, tc3, (4 * g + j) * P:(4 * g + j + 1) * P],
                            ident_t[0:rows, 0:rows],
                        )
                    nc.scalar.activation(
                        xT_t[:, 4 * g:4 * g + 4, tc3 * P:tc3 * P + rows],
                        pt[:, :, 0:rows],
                        mybir.ActivationFunctionType.Copy,
                    )

            # ---- v projection (natural orientation, ext rows) ----
            v_t = sb.tile([P, NKB, D], bf16, tag="v")
            for tc3 in range(NKB):
                rows = P if tc3 < 2 else TEXT - 2 * P
                for half in range(2):
                    pv = pp.tile([P, 512], f32, tag="pp")
                    for k in range(KCH):
                        nc.tensor.matmul(
                            pv[0:rows, :],
                            xT_t[:, k, tc3 * P:tc3 * P + rows],
                            wv_t[:, k, half * 512:(half + 1) * 512],
                            start=(k == 0), stop=(k == KCH - 1),
                        )
                    nc.scalar.activation(
                        v_t[0:rows, tc3, half * 512:(half + 1) * 512],
                        pv[0:rows, :], mybir.ActivationFunctionType.Copy)

            # ---- interleaved q/k projection chunks + attention pairs ----
            # Chunk c of qT/kT holds exactly head pair c, so pair c's
            # attention is emitted right after its projection chunk; the
            # next chunk's matmuls keep the PE busy while sigmoid/mask for
            # the current pair run on ACT/DVE.
            qT = sb.tile([P, MCH, TLOC], bf16, tag="qT")
            kT = sb.tile([P, MCH, TEXT], bf16, tag="kT")

            def proj_chunk(wt, m, ncols, col_off, cos_t, sin_t, outT):
                pq = pp.tile([P, 512], f32, tag="pp")
                for k in range(KCH):
                    nc.tensor.matmul(
                        pq[:, 0:ncols],
                        wt[:, k, m * P:(m + 1) * P],
                        xT_t[:, k, col_off:col_off + ncols],
                        start=(k == 0), stop=(k == KCH - 1),
                    )
                a_t = sba.tile([P, TEXT], bf16, tag="a")
                nc.scalar.activation(a_t[:, 0:ncols], pq[:, 0:ncols],
                                     mybir.ActivationFunctionType.Copy)
                pr = prot.tile([P, 512], f32, tag="rot")
                nc.tensor.matmul(pr[:, 0:ncols], rot_t[:], a_t[:, 0:ncols],
                                 start=True, stop=True)
                t1 = sba.tile([P, TEXT], bf16, tag="t1")
                nc.vector.tensor_tensor(t1[:, 0:ncols], a_t[:, 0:ncols],
                                        cos_t[:], op=mybir.AluOpType.mult)
                t2 = sba.tile([P, TEXT], bf16, tag="t2")
                nc.vector.tensor_tensor(t2[:, 0:ncols], pr[:, 0:ncols],
                                        sin_t[:], op=mybir.AluOpType.mult)
                nc.vector.tensor_tensor(outT[:, m, :], t1[:, 0:ncols],
                                        t2[:, 0:ncols], op=mybir.AluOpType.add)

            msgf = sb.tile([P, MCH, TLOC], f32, tag="msgf")
            msgb = sb.tile([P, MCH, TLOC], bf16, tag="msgb")
            pmass_t = pms.tile([16, TLOC], f32, tag="pmass")
            n_mass = 2 * NKB * NPAIR
            mstate = {"i": 0}
            taus = {}

            def emit_msg(c):
                tau_t = taus.pop(c)
                pm_t = pm.tile([P, TLOC], f32, tag="pm", name=f"pm{c}")
                for hh in range(2):
                    h = 2 * c + hh
                    for j, (kb, tlo, thi) in enumerate(
                            [(1, 0, TLOC), (0, 0, P), (2, 192, TLOC)]):
                        krows = P if kb < 2 else TEXT - 2 * P
                        nc.tensor.matmul(
                            pm_t[hh * HD:(hh + 1) * HD, tlo:thi],
                            v_t[0:krows, kb, h * HD:(h + 1) * HD],
                            tau_t[0:krows, kb, hh, tlo:thi],
                            start=(j == 0), stop=(j == 2),
                            tile_position=(0, hh * HD),
                            skip_group_check=True,
                        )
                        if stage >= 40:
                            nc.tensor.matmul(
                                pmass_t[:, tlo:thi],
                                emask_t[0:krows, h, :],
                                tau_t[0:krows, kb, hh, tlo:thi],
                                start=(mstate["i"] == 0),
                                stop=(mstate["i"] == n_mass - 1),
                                skip_group_check=True,
                            )
                            mstate["i"] += 1
                nc.vector.tensor_copy(msgf[:, c, :], pm_t[:])

            for c in range(NPAIR):
                proj_chunk(wq_t, c, TLOC, WN, cq_t, sq_t, qT)
                proj_chunk(wk_t, c, TEXT, 0, ck_t, sk_t, kT)
                if stage >= 30 and c > 0:
                    emit_msg(c - 1)
                if stage < 11:
                    continue
                ps_t = ps.tile([P, NKB, 2, TLOC], f32, tag="ps", name=f"ps{c}")
                for hh in range(2):
                    po = hh * HD
                    for kb in range(NKB):
                        krows = P if kb < 2 else TEXT - 2 * P
                        tlo = 0 if kb < 2 else 192
                        nc.tensor.matmul(
                            ps_t[0:krows, kb, hh, tlo:TLOC],
                            kT[po:po + HD, c, kb * P:kb * P + krows],
                            qT[po:po + HD, c, tlo:TLOC],
                            start=True, stop=True,
                        )
                tau_t = sbt.tile([P, NKB, 2, TLOC], bf16, tag="tau",
                                 name=f"tau{c}")
                taus[c] = tau_t
                if stage >= 12:
                    nc.scalar.activation(
                        tau_t[:, 0:2, :, :], ps_t[:, 0:2, :, :],
                        mybir.ActivationFunctionType.Sigmoid)
                    nc.scalar.activation(
                        tau_t[0:HD, 2, :, 192:TLOC],
                        ps_t[0:HD, 2, :, 192:TLOC],
                        mybir.ActivationFunctionType.Sigmoid)
                if stage >= 20:
                    nc.vector.tensor_tensor(tau_t[:, 1, :, :],
                                            tau_t[:, 1, :, :],
                                            mask_t[:, 1, :, :],
                                            op=mybir.AluOpType.mult)
                    nc.vector.tensor_tensor(tau_t[:, 0, :, 0:P],
                                            tau_t[:, 0, :, 0:P],
                                            mask_t[:, 0, :, 0:P],
                                            op=mybir.AluOpType.mult)
                    nc.vector.tensor_tensor(tau_t[0:HD, 2, :, 192:TLOC],
                                            tau_t[0:HD, 2, :, 192:TLOC],
                                            mask_t[0:HD, 2, :, 192:TLOC],
                                            op=mybir.AluOpType.mult)
            if stage >= 30:
                emit_msg(NPAIR - 1)

            # ---- tau-mass normalization ----
            rinv_t = sb.tile([16, TLOC], bf16, tag="rinv")
            if stage >= 40:
                mass_sb = sb.tile([16, TLOC], f32, tag="mass")
                nc.vector.tensor_scalar_max(mass_sb[:], pmass_t[:], 1e-6)
                rinvf = sb.tile([16, TLOC], f32, tag="rinvf")
                nc.vector.reciprocal_approx_fast(rinvf[:], mass_sb[:])
                with nc.allow_low_precision(reason="bf16 1/mass is fine"):
                    nc.vector.tensor_copy(rinv_t[:], rinvf[:])
            else:
                nc.vector.memset(rinv_t[:], 1.0)
            if stage < 30:
                nc.vector.memset(msgf[:], 0.01)
            for c in range(NPAIR):
                prv = pp.tile([P, 512], f32, tag="pp", name=f"prv{c}")
                nc.tensor.matmul(prv[:, 0:TLOC], esel_t[:, c, :], rinv_t[:],
                                 start=True, stop=True)
                with nc.allow_low_precision(reason="bf16 msg is fine"):
                    nc.vector.tensor_tensor(msgb[:, c, :], msgf[:, c, :],
                                            prv[:, 0:TLOC],
                                            op=mybir.AluOpType.mult)

            # ---- output projection + residual + rms norm ----
            for t2 in range(2):
                z_t = sbz.tile([P, D], f32, tag="z")
                for half in range(2):
                    pz = pp.tile([P, 512], f32, tag="pp")
                    for k in range(KCH):
                        nc.tensor.matmul(
                            pz[:, :],
                            msgb[:, k, t2 * P:(t2 + 1) * P],
                            wo_t[:, k, half * 512:(half + 1) * 512],
                            start=(k == 0), stop=(k == KCH - 1),
                        )
                    nc.vector.tensor_tensor(
                        z_t[:, half * 512:(half + 1) * 512],
                        pz[:, :], xo_t[:, t2, half * 512:(half + 1) * 512],
                        op=mybir.AluOpType.add)
                z2 = sbz.tile([P, D], f32, tag="zs", name="z2")
                ssq = sbz.tile([P, 1], f32, tag="ssq")
                nc.vector.scalar_tensor_tensor(
                    z2[:], z_t[:], 1.0, z_t[:],
                    op0=mybir.AluOpType.bypass, op1=mybir.AluOpType.mult,
                    accum_out=ssq[:])
                nc.vector.tensor_scalar(ssq[:], ssq[:], D * EPS, None,
                                        op0=mybir.AluOpType.add)
                sroot = sbz.tile([P, 1], f32, tag="sroot")
                nc.scalar.activation(sroot[:], ssq[:],
                                     mybir.ActivationFunctionType.Sqrt)
                rinv2 = sbz.tile([P, 1], f32, tag="rinv2")
                nc.vector.reciprocal(rinv2[:], sroot[:])
                out_t = sbz.tile([P, D], f32, tag="zs", name="out_t")
                # out = (z * rinv2) * nsc in one fused DVE op
                nc.vector.scalar_tensor_tensor(
                    out_t[:], z_t[:], rinv2[:], nsc_t[:],
                    op0=mybir.AluOpType.mult, op1=mybir.AluOpType.mult)
                nc.sync.dma_start(y_d[t2 * P:(t2 + 1) * P, :], out_t[:])

    nc.compile()
    return nc


def _host_tables():
    """Core-independent constant inputs."""
    half = HD // 2
    bft = mybir.dt.np(bf16)
    ident = np.eye(P, dtype=np.float32)
    # Rot = blockdiag(J, J) with J = [[0, -I32], [I32, 0]] on 64-row groups
    rot = np.zeros((P, P), dtype=np.float32)
    for g in range(2):
        o = g * 64
        for r in range(half):
            rot[o + r, o + half + r] = -1.0
            rot[o + half + r, o + r] = 1.0
    rotT = rot.T.copy().astype(bft)
    emask = np.zeros((P, H, H), dtype=np.float32)
    for h in range(H):
        emask[:, h, h] = 1.0
    esel = np.zeros((H, MCH, P), dtype=np.float32)
    for c in range(MCH):
        esel[2 * c, c, 0:HD] = 1.0
        esel[2 * c + 1, c, HD:P] = 1.0
    return ident, rotT, emask.astype(bft), esel.astype(bft)


def _trig(positions: np.ndarray, scale: float):
    """cos/sin tables tiled to [128, len(positions)]; cos bf16, sin f32."""
    half = HD // 2
    bft = mybir.dt.np(bf16)
    theta = 1.0 / (ROPE_BASE ** (np.arange(half, dtype=np.float64) / half))
    freqs = positions[:, None].astype(np.float64) * theta[None, :]  # [n, 32]
    c = (np.cos(freqs).T * scale).astype(np.float32)  # [32, n]
    s = (np.sin(freqs).T * scale).astype(np.float32)
    return np.tile(c, (4, 1)).astype(bft), np.tile(s, (4, 1))


def _masks(core: int) -> np.ndarray:
    """[P, NKB, 2, TLOC] bf16: mask[p, kb, :, t] = 1 iff key ext row
    128kb+p is in query t's window (and causally valid for core 0)."""
    m = np.zeros((NKB, P, TLOC), dtype=np.float32)
    t = np.arange(TLOC)[None, :]
    for kb in range(NKB):
        j = np.arange(P)[:, None]
        w = 128 * kb + j - t
        valid = (w >= 0) & (w < WN)
        if core == 0:
            valid &= (128 * kb + j) >= WN
        m[kb] = valid.astype(np.float32)
    m2 = np.broadcast_to(m.transpose(1, 0, 2)[:, :, None, :],
                         (P, NKB, 2, TLOC))
    return np.ascontiguousarray(m2).astype(mybir.dt.np(bf16))


_CACHE = {}


def _make_runner(nc):
    """Persistent sharded-jit executor over the 8 cores (mirrors
    bass2jax.run_bass_via_pjrt's multi-core path, but reusable so repeat
    calls skip retracing/recompilation)."""
    import jax
    from jax.experimental.shard_map import shard_map
    from jax.sharding import Mesh, PartitionSpec
    from concourse import bass2jax

    bass2jax.install_neuronx_cc_hook()
    partition_name = (nc.partition_id_tensor.name
                      if nc.partition_id_tensor else None)
    in_names, out_names, out_avals = [], [], []
    for alloc in nc.m.functions[0].allocations:
        if not isinstance(alloc, mybir.MemoryLocationSet):
            continue
        if alloc.kind not in ("ExternalInput", "ExternalOutput"):
            continue
        name = alloc.memorylocations[0].name
        if alloc.kind == "ExternalInput":
            if name != partition_name:
                in_names.append(name)
        else:
            out_names.append(name)
            out_avals.append(jax.core.ShapedArray(
                tuple(alloc.tensor_shape), mybir.dt.np(alloc.dtype)))
    n_params, n_outs = len(in_names), len(out_names)
    bind_names = in_names + out_names + (
        [partition_name] if partition_name else [])

    def _body(*args):
        operands = list(args)
        if partition_name is not None:
            operands.append(bass2jax.partition_id_tensor())
        outs = bass2jax._bass_exec_p.bind(
            *operands,
            out_avals=tuple(out_avals),
            in_names=tuple(bind_names),
            out_names=tuple(out_names),
            lowering_input_output_aliases=(),
            sim_require_finite=True,
            sim_require_nnan=True,
            nc=nc,
        )
        return tuple(outs)

    devices = jax.devices()[:NCORES]
    mesh = Mesh(np.asarray(devices), ("core",))
    sharded = jax.jit(
        shard_map(_body, mesh=mesh,
                  in_specs=(PartitionSpec("core"),) * (n_params + n_outs),
                  out_specs=(PartitionSpec("core"),) * n_outs,
                  check_rep=False),
        donate_argnums=tuple(range(n_params, n_params + n_outs)),
        keep_unused=True)

    def run(in_maps):
        concat_in = [np.concatenate([m[name] for m in in_maps], axis=0)
                     for name in in_names]
        zeros = [np.zeros((NCORES * a.shape[0], *a.shape[1:]), a.dtype)
                 for a in out_avals]
        out_arrs = sharded(*concat_in, *zeros)
        return [
            {name: np.asarray(out_arrs[i]).reshape(
                NCORES, *out_avals[i].shape)[c]
             for i, name in enumerate(out_names)}
            for c in range(NCORES)
        ]

    run.sharded = sharded
    run.in_names = in_names
    run.out_names = out_names
    run.out_avals = out_avals
    return run


def _in_maps(x, wq, wk, wv, wo, norm_scale):
    bft = mybir.dt.np(bf16)
    ident, rotT, emask, esel = _host_tables()
    wq_b = np.asarray(wq, dtype=np.float32).astype(bft)
    wk_b = np.asarray(wk, dtype=np.float32).astype(bft)
    wv_b = np.asarray(wv, dtype=np.float32).astype(bft)
    wo_b = np.asarray(wo, dtype=np.float32).astype(bft)
    nsc = np.ascontiguousarray(
        np.broadcast_to(np.asarray(norm_scale, dtype=np.float32)
                        * np.float32(np.sqrt(D)), (P, D)))

    xf = np.asarray(x, dtype=np.float32).reshape(T, D)
    in_maps = []
    for c in range(NCORES):
        t0 = c * TLOC
        x_halo = np.zeros((TEXT, D), dtype=np.float32)
        lo = max(0, t0 - WN)
        x_halo[WN - (t0 - lo):] = xf[lo:t0 + TLOC]
        cosq, sinq = _trig(np.arange(t0, t0 + TLOC), 1.0 / 8.0)
        cosk, sink = _trig(np.arange(t0 - WN, t0 + TLOC), 1.0)
        in_maps.append({
            "x_halo": x_halo,
            "wq_b": wq_b, "wk_b": wk_b, "wv_b": wv_b, "wo_b": wo_b,
            "ident": ident, "rotT": rotT,
            "cosq": cosq, "sinq": sinq, "cosk": cosk, "sink": sink,
            "masks2": _masks(c), "emaskT": emask, "esel": esel,
            "norm_scale": nsc,
        })
    return in_maps


def kernel(x, wq, wk, wv, wo, norm_scale):
    if "nc" not in _CACHE:
        _CACHE["nc"] = _build_program()
        _CACHE["runner"] = _make_runner(_CACHE["nc"])
    nc = _CACHE["nc"]
    in_maps = _in_maps(x, wq, wk, wv, wo, norm_scale)
    _CACHE["last_in_maps"] = in_maps
    if "first_done" not in _CACHE:
        res = bass_utils.run_bass_kernel_spmd(
            nc, in_maps, core_ids=list(range(NCORES)))
        results = res.results
        _CACHE["first_done"] = True
    else:
        results = _CACHE["runner"](in_maps)
    out = np.empty((1, T, D), dtype=np.float32)
    for c in range(NCORES):
        out[0, c * TLOC:(c + 1) * TLOC] = results[c]["y"]
    return out
